# revision 34
# baseline (speedup 1.0000x reference)
"""GQA attention block (B=2,T=2048,E=2048,H=16,KV=4) on 8 trn2 NeuronCores.

Sharding: core c -> batch b=c//4, kv-group g=c%4 (q-heads 4g..4g+3, kv head g).
Each core computes its 4 heads end-to-end plus the partial output projection
(Wo rows for its heads); host sums the 4 partials per batch and adds bias.

v4 schedule/layout (single continuous PE stream):
  - 3-deep software-pipelined attention: QK(tk) runs two tiles ahead of
    PV(tk-2), hiding the ~900ns exp latency; one "filler" matmul (next
    chunk's projections / previous chunk's out-projection) is pumped into
    the PE queue per attention tile so the PE never idles and the HAM
    clock gate stays at 8/8 (2.4 GHz).
  - Warmup dummy matmuls during the initial input-DMA wait warm the HAM
    before real work; chunk-0 x is DMA'd in 4 bands so the first
    projection chain starts ~1.5us in.
  - Softmax denominators: bf16 accumulation on DVE, ones-matmul
    broadcast on PE (gpsimd is unusable: Q7 lib reload per op switch,
    4.4us per custom op), reciprocal_approx_fast + tensor_mul divides.
  - RoPE: pre-rotation values kept in f16 (reference casts to bf16 only
    AFTER rotation - quantizing before it was the dominant error source);
    half-swap via a f16 permutation matmul on PE; bias-add+PSUM-read on
    DVE tensor_scalar_add (keeps ACT exp-only); cos/sin tables in f16.
  - Causal diagonal at 128-column granularity (tri mask multiply on DVE).
  - V projection computes V^T (wv stationary), XBAR transpose-DMA into
    [s,d] tiles for the PV matmul.
  - Output stored f16; host accumulates partials in f64.
"""

import numpy as np

for _p in ("/opt/trn_rl_repo", "/root/.axon_site/_ro/trn_rl_repo"):
    import sys

    if _p not in sys.path:
        sys.path.insert(0, _p)

import ml_dtypes
from collections import deque
from contextlib import ExitStack

import concourse.bass as bass
import concourse.bass_isa as bass_isa
import concourse.mybir as mybir
import concourse.tile as tile
from concourse import bacc
from concourse.bass_utils import run_bass_kernel_spmd

F32 = mybir.dt.float32
BF16 = mybir.dt.bfloat16
F16 = mybir.dt.float16
T = 2048
E = 2048
HD = 128
NQH = 4          # q heads per core
KT = E // 128    # 16 k-tiles over embed
NC = T // 512    # 4 512-chunks over time
SCALE = float(E) ** -0.5

_program = None
LAST_EXEC_NS = None
LAST_TRACE = None
LAST_PROFILE_JSON = None


def _build_program():
    nc = bacc.Bacc("TRN2", target_bir_lowering=False, debug=False, num_devices=8)
    # Big inputs stay row-major: the resulting many-small-descriptor gather
    # DMAs scatter across all 16 DMA engines (a single contiguous transfer
    # serializes onto one engine at ~60GB/s - measured).
    xT_d = nc.declare_dram_parameter("xT", [E, T], F16, isOutput=False)
    wq_d = nc.declare_dram_parameter("wq", [E, NQH * HD], F16, isOutput=False)
    wk_d = nc.declare_dram_parameter("wk", [E, HD], F16, isOutput=False)
    wv_d = nc.declare_dram_parameter("wv", [E, HD], F16, isOutput=False)
    wo_d = nc.declare_dram_parameter("wo", [NQH * HD, E], BF16, isOutput=False)
    ct_d = nc.declare_dram_parameter("ct", [HD, T], F16, isOutput=False)
    st_d = nc.declare_dram_parameter("st", [HD, T], F16, isOutput=False)
    tri_d = nc.declare_dram_parameter("tri", [HD, HD], BF16, isOutput=False)
    psw_d = nc.declare_dram_parameter("psw", [HD, HD], F16, isOutput=False)
    bq_d = nc.declare_dram_parameter("bq", [HD, NQH], F32, isOutput=False)
    bk_d = nc.declare_dram_parameter("bk", [HD, 1], F32, isOutput=False)
    out_d = nc.declare_dram_parameter("out", [T, E], F16, isOutput=True)

    with tile.TileContext(nc) as tc, ExitStack() as ctx:
        consts = ctx.enter_context(tc.tile_pool(name="consts", bufs=1))
        rope = ctx.enter_context(tc.tile_pool(name="rope", bufs=2))
        vsp = ctx.enter_context(tc.tile_pool(name="vsp", bufs=2))
        ptp = ctx.enter_context(tc.tile_pool(name="ptp", bufs=4))
        accp = ctx.enter_context(tc.tile_pool(name="accp", bufs=2))
        rinvp = ctx.enter_context(tc.tile_pool(name="rinvp", bufs=2))
        otp = ctx.enter_context(tc.tile_pool(name="otp", bufs=12))
        outp = ctx.enter_context(tc.tile_pool(name="outp", bufs=2))
        # PSUM: pss 3 + scr(qsw/psb) 1 + psot 2 + psP 2 = 8 banks exactly
        psS = ctx.enter_context(tc.tile_pool(name="psS", bufs=3, space=bass.MemorySpace.PSUM))
        psOT = ctx.enter_context(tc.tile_pool(name="psOT", bufs=2, space=bass.MemorySpace.PSUM))
        psP = ctx.enter_context(tc.tile_pool(name="psP", bufs=2, space=bass.MemorySpace.PSUM))

        # ---- persistent tiles ---------------------------------------------
        wk = consts.tile([128, KT * HD], F16, tag="wk", name="wk")
        wv = consts.tile([128, KT * HD], F16, tag="wv", name="wv")
        wq = consts.tile([128, KT * NQH * HD], F16, tag="wq", name="wq")
        wo = consts.tile([128, NQH * E], BF16, tag="wo", name="wo")
        xb0 = [consts.tile([128, 4 * 512], F16, tag=f"xb0_{b}", name=f"xb0_{b}")
               for b in range(4)]
        xtc = [None] + [consts.tile([128, KT * 512], F16, tag=f"xtc{c}", name=f"xtc{c}")
                        for c in range(1, NC)]
        ctc = [consts.tile([128, 512], F16, tag=f"ctc{c}", name=f"ctc{c}")
               for c in range(NC)]
        stc = [consts.tile([128, 512], F16, tag=f"stc{c}", name=f"stc{c}")
               for c in range(NC)]
        tri = consts.tile([128, 128], BF16, tag="tri", name="tri")
        psw = consts.tile([128, 128], F16, tag="psw", name="psw")
        bq_t = consts.tile([HD, NQH], F32, tag="bq", name="bq_t")
        bk_t = consts.tile([HD, 1], F32, tag="bk", name="bk_t")
        zeros = consts.tile([128, 512], BF16, tag="zeros", name="zeros")
        ones128 = consts.tile([128, 128], BF16, tag="ones", name="ones128")
        nc.vector.memset(ones128[:], 1.0)

        qTc = [[consts.tile([128, 512], BF16, tag=f"qT{h}_{c}", name=f"qT{h}_{c}")
                for c in range(NC)] for h in range(NQH)]
        kTc = [consts.tile([128, 512], BF16, tag=f"kT{c}", name=f"kT{c}")
               for c in range(NC)]
        vA = [consts.tile([128, 128], BF16, tag=f"vA{t}", name=f"vA{t}")
              for t in range(4 * NC)]

        def xsl(c, k):
            # k-tile [128, 512] of chunk c's transposed x
            if c == 0:
                return xb0[k // 4][:, (k % 4) * 512:(k % 4 + 1) * 512]
            return xtc[c][:, k * 512:(k + 1) * 512]

        def split_rows(src_ap, p=128):
            # [(k p), f] -> [p, k, f]: one DMA that deposits each 128-row
            # band k into its own column block of the destination tile.
            return src_ap.rearrange("(k p) f -> p k f", p=p)

        # ---- PE warmup (HAM un-throttle) before input DMAs land -----------
        nc.vector.memset(zeros[:], 0.0)

        def warm_mm():
            warm = psP.tile([128, 512], F32, tag="psP", name="warm")
            nc.tensor.matmul(warm[:], zeros[:, 0:128], zeros[:], start=True, stop=True)

        for i in range(10):
            warm_mm()

        # ---- input DMA issue order (first-needed first) -------------------
        # Spread across three idle issue queues: sync=x data,
        # gpsimd=small weights/tables, scalar=wq/wo (big weights).
        nc.gpsimd.dma_start(wk[:], split_rows(wk_d[:, :]))
        nc.gpsimd.dma_start(wv[:], split_rows(wv_d[:, :]))
        nc.scalar.dma_start(wq[:], split_rows(wq_d[:, :]))
        nc.scalar.dma_start(wo[:], split_rows(wo_d[:, :]))
        nc.gpsimd.dma_start(ctc[0][:], ct_d[:, 0:512])
        nc.gpsimd.dma_start(stc[0][:], st_d[:, 0:512])
        nc.gpsimd.dma_start(psw[:], psw_d[:, :])
        nc.gpsimd.dma_start(bk_t[:], bk_d[:, :])
        nc.gpsimd.dma_start(bq_t[:], bq_d[:, :])
        nc.gpsimd.dma_start(tri[:], tri_d[:, :])
        for b in range(4):
            nc.sync.dma_start(xb0[b][:], split_rows(xT_d[b * 512:(b + 1) * 512, 0:512]))
        for c in range(1, NC):
            nc.sync.dma_start(xtc[c][:], split_rows(xT_d[:, c * 512:(c + 1) * 512]))
            nc.gpsimd.dma_start(ctc[c][:], ct_d[:, c * 512:(c + 1) * 512])
            nc.gpsimd.dma_start(stc[c][:], st_d[:, c * 512:(c + 1) * 512])

        # ---- filler scheduler ---------------------------------------------
        prereq = deque()    # proj units for the NEXT chunk: must drain before it
        optional = deque()  # outproj units: drain opportunistically

        def pump(n):
            k = 0
            while k < n:
                q = prereq if prereq else optional
                if not q:
                    return k
                try:
                    next(q[0])
                    k += 1
                except StopIteration:
                    q.popleft()
            return k

        def drain(q):
            while q:
                try:
                    next(q[0])
                except StopIteration:
                    q.popleft()

        # ---- projection units (generators yielding after each PE instr) ---
        def gen_proj(c, w_col, dst, bias_ap, kind, warm_pad=0):
            ps = psP.tile([128, 512], F32, tag="psP", name=f"ps_{kind}{c}")
            for k in range(KT):
                nc.tensor.matmul(ps[:], w_col(k), xsl(c, k),
                                 start=(k == 0), stop=(k == KT - 1))
                if warm_pad and k < KT - 1:
                    # keep HAM warm while DMA-paced: dep-free dummy matmuls
                    # into the scratch bank (free during chunk-0 V proj)
                    for _ in range(warm_pad):
                        wt = psS.tile([128, 512], F32, tag="scr", bufs=1,
                                      name="warmp")
                        nc.tensor.matmul(wt[:], zeros[:, 0:128], zeros[:],
                                         start=True, stop=True)
                yield
            if kind == "v":
                vsb = vsp.tile([128, 512], BF16, tag="vsb", name="vsb")
                nc.scalar.copy(vsb[:], ps[:])
                for tt in range(4):
                    nc.sync.dma_start(
                        vA[4 * c + tt][:], vsb[:, tt * 128:(tt + 1) * 128],
                        transpose=True)
            else:
                # rope: qsb = ps + bias (f16, on DVE); half-swap on PE;
                # dst = qsb*cos (pool) + qsw*sin (dve), summed on pool.
                qsb = rope.tile([128, 512], F16, tag="qsb", name="qsb")
                nc.vector.tensor_scalar_add(qsb[:], ps[:], bias_ap)
                qsw = psS.tile([128, 512], F32, tag="scr", bufs=1, name="qsw")
                nc.tensor.matmul(qsw[:], psw[:], qsb[:], start=True, stop=True)
                yield
                t1 = rope.tile([128, 512], F32, tag="t1", name="t1")
                nc.vector.tensor_mul(t1[:], qsb[:], ctc[c][:])
                t2 = rope.tile([128, 512], F32, tag="t2", name="t2")
                nc.vector.tensor_mul(t2[:], qsw[:], stc[c][:])
                nc.vector.tensor_add(dst[:], t1[:], t2[:])

        def enqueue_proj(c):
            # V first: its transpose-DMA chain has the longest latency tail
            prereq.append(gen_proj(
                c, lambda k: wv[:, k * HD:(k + 1) * HD], None, None, "v",
                warm_pad=2 if c == 0 else 0))
            prereq.append(gen_proj(
                c, lambda k: wk[:, k * HD:(k + 1) * HD], kTc[c], bk_t[:, 0:1], "k"))
            for h in range(NQH):
                prereq.append(gen_proj(
                    c,
                    lambda k, h=h: wq[:, k * 512 + h * HD:k * 512 + (h + 1) * HD],
                    qTc[h][c], bq_t[:, h:h + 1], "q"))

        # ---- out-projection units -----------------------------------------
        def gen_outproj(qc, ots):
            for i in range(4):
                osb = outp.tile([128, E], F16, tag="osb", name="osb")
                rows = slice((qc * 4 + i) * 128, (qc * 4 + i + 1) * 128)
                for e in range(4):
                    psf = psP.tile([128, 512], F32, tag="psP", name="psf")
                    for h in range(NQH):
                        nc.tensor.matmul(
                            psf[:], ots[h][:, i * 128:(i + 1) * 128],
                            wo[:, h * E + e * 512:h * E + (e + 1) * 512],
                            start=(h == 0), stop=(h == NQH - 1))
                        yield
                    esl = slice(e * 512, (e + 1) * 512)
                    if e % 2 == 0:
                        nc.vector.tensor_copy(osb[:, esl], psf[:])
                    else:
                        nc.scalar.copy(osb[:, esl], psf[:])
                    nc.gpsimd.dma_start(out_d[rows, esl], osb[:, esl])

        # ---- attention for one 512-query chunk (per head, 3-deep pipe) ----
        def attention(qc):
            Tt = 4 * qc + 4
            ots = []
            for h in range(NQH):
                psot = psOT.tile([128, 512], F32, tag="psot", name="psot")
                acc = accp.tile([128, 512], BF16, tag="acc", name="acc")
                pts = [None] * Tt

                def emit_qk(tk, h=h, acc=acc, pts=pts):
                    pss = psS.tile([128, 512], F32, tag="pss", name="pss")
                    pt = ptp.tile([128, 512], BF16, tag="pt", name="pt")
                    if tk < 4 * qc:
                        nc.tensor.matmul(
                            pss[:], kTc[tk // 4][:, (tk % 4) * 128:(tk % 4 + 1) * 128],
                            qTc[h][qc][:], start=True, stop=True)
                        nc.scalar.activation(
                            pt[:], pss[:], mybir.ActivationFunctionType.Exp)
                        if tk == 0:
                            nc.vector.tensor_copy(acc[:], pt[:])
                        else:
                            nc.vector.tensor_add(acc[:], acc[:], pt[:])
                    else:
                        j = tk - 4 * qc
                        sl = slice(j * 128, 512)
                        dsl = slice(j * 128, (j + 1) * 128)
                        nc.tensor.matmul(
                            pss[:, sl], kTc[qc][:, j * 128:(j + 1) * 128],
                            qTc[h][qc][:, sl], start=True, stop=True)
                        nc.scalar.activation(
                            pt[:, sl], pss[:, sl], mybir.ActivationFunctionType.Exp)
                        nc.vector.tensor_mul(pt[:, dsl], pt[:, dsl], tri[:])
                        if tk == 0:
                            nc.vector.tensor_copy(acc[:], pt[:])
                        else:
                            nc.vector.tensor_add(acc[:, sl], acc[:, sl], pt[:, sl])
                    pts[tk] = pt

                def emit_pv(tk, psot=psot, pts=pts):
                    pt = pts[tk]
                    start = (tk == 0)
                    stop = (tk == Tt - 1)
                    if tk < 4 * qc:
                        nc.tensor.matmul(psot[:], vA[tk][:], pt[:],
                                         start=start, stop=stop)
                    else:
                        j = tk - 4 * qc
                        sl = slice(j * 128, 512)
                        nc.tensor.matmul(psot[:, sl], vA[tk][:], pt[:, sl],
                                         start=start, stop=stop)
                    pts[tk] = None

                emit_qk(0)
                if Tt > 1:
                    emit_qk(1)
                for tk in range(2, Tt):
                    emit_qk(tk)
                    pump(1)
                    emit_pv(tk - 2)
                pump(1)
                if Tt > 1:
                    emit_pv(Tt - 2)
                pump(1)
                emit_pv(Tt - 1)

                # finalize head: denominator (PE ones-matmul broadcast) + divide
                psb = psS.tile([128, 512], F32, tag="scr", bufs=1, name="psb")
                nc.tensor.matmul(psb[:], ones128[:], acc[:], start=True, stop=True)
                rinv = rinvp.tile([128, 512], F32, tag="rinv", name="rinv")
                nc.vector.reciprocal_approx_fast(out=rinv[:], in_=psb[:])
                ot = otp.tile([128, 512], BF16, tag="ot", name="ot")
                nc.vector.tensor_mul(ot[:], psot[:], rinv[:])
                ots.append(ot)
            return ots

        # ---- schedule ------------------------------------------------------
        # chunk-0 projections run in the open (nothing to hide them under)
        enqueue_proj(0)
        drain(prereq)
        for qc in range(NC):
            if qc + 1 < NC:
                enqueue_proj(qc + 1)
            ots = attention(qc)
            optional.append(gen_outproj(qc, ots))
            drain(prereq)
        drain(optional)
    nc.compile()
    return nc


def _rope_tables():
    # quirk: freq exponent uses full n_embed then slices to head_dim//2
    freqs = 10000.0 ** (-(np.arange(0, E, 2, dtype=np.float64) / E))[:HD // 2]
    t = np.arange(T, dtype=np.float64)
    ang = np.outer(freqs, t)                      # [64, T]
    ct = np.empty((HD, T), np.float32)
    st = np.empty((HD, T), np.float32)
    ct[:64] = np.cos(ang)
    ct[64:] = np.cos(ang)
    st[:64] = -np.sin(ang)
    st[64:] = np.sin(ang)
    return ct, st


def kernel(x, Wq, bq, Wk, bk, Wv, bv, Wo, bo):
    global _program, LAST_EXEC_NS, LAST_TRACE, LAST_PROFILE_JSON
    x = np.asarray(x, np.float32)
    Wq, bq = np.asarray(Wq, np.float32), np.asarray(bq, np.float32)
    Wk, bk = np.asarray(Wk, np.float32), np.asarray(bk, np.float32)
    Wv, bv = np.asarray(Wv, np.float32), np.asarray(bv, np.float32)
    Wo, bo = np.asarray(Wo, np.float32), np.asarray(bo, np.float32)
    bf = ml_dtypes.bfloat16

    if _program is None:
        _program = _build_program()

    perm = np.concatenate([np.arange(0, HD, 2), np.arange(1, HD, 2)])
    ct, st = _rope_tables()
    tri = (np.arange(128)[None, :] >= np.arange(128)[:, None]).astype(np.float32)
    psw = np.zeros((128, 128), np.float32)
    psw[(np.arange(128) + 64) % 128, np.arange(128)] = 1.0

    xT = [np.ascontiguousarray(x[b].T).astype(np.float16) for b in range(2)]
    in_maps = []
    for c in range(8):
        b, g = divmod(c, 4)
        qcols = np.concatenate([(4 * g + h) * HD + perm for h in range(NQH)])
        kcols = g * HD + perm
        vcols = np.arange(g * HD, (g + 1) * HD)
        in_maps.append({
            "xT": xT[b],
            "wq": Wq[:, qcols].astype(np.float16),
            "wk": Wk[:, kcols].astype(np.float16),
            "wv": Wv[:, vcols].astype(np.float16),
            "wo": (Wo[g * 512:(g + 1) * 512, :] * SCALE).astype(bf),
            "ct": ct.astype(np.float16),
            "st": st.astype(np.float16),
            "tri": tri.astype(bf),
            "psw": psw.astype(np.float16),
            "bq": np.ascontiguousarray(
                bq[np.concatenate([(4 * g + h) * HD + perm for h in range(NQH)])]
                .reshape(NQH, HD).T).astype(np.float32),
            "bk": bk[kcols].reshape(HD, 1).astype(np.float32),
        })

    import time
    t0 = time.time()
    res = run_bass_kernel_spmd(_program, in_maps, list(range(8)))
    t1 = time.time()
    LAST_EXEC_NS = res.exec_time_ns
    if res.instructions_and_trace is not None:
        LAST_TRACE = res.instructions_and_trace[1]
    LAST_PROFILE_JSON = res.profile_json
    if LAST_EXEC_NS is None:
        LAST_EXEC_NS = int((t1 - t0) * 1e9)  # wall time incl. H2D (upper bound)

    out = np.zeros((2, T, E), np.float64)
    for c in range(8):
        out[c // 4] += np.asarray(res.results[c]["out"], np.float64)
    # bv folded: after softmax each row sums to 1, scaled by SCALE inside Wo
    obias = np.repeat(bv.astype(np.float64).reshape(4, HD), 4, axis=0).reshape(-1)
    bo_eff = bo.astype(np.float64) + SCALE * (obias @ Wo.astype(np.float64))
    out += bo_eff[None, None, :]
    return out.astype(np.float32)


# revision 35
# speedup vs baseline: 1.0295x; 1.0295x over previous
"""GQA attention block (B=2,T=2048,E=2048,H=16,KV=4) on 8 trn2 NeuronCores.

Sharding: core c -> batch b=c//4, kv-group g=c%4 (q-heads 4g..4g+3, kv head g).
Each core computes its 4 heads end-to-end plus the partial output projection
(Wo rows for its heads); host sums the 4 partials per batch and adds bias.

v4 schedule/layout (single continuous PE stream):
  - 3-deep software-pipelined attention: QK(tk) runs two tiles ahead of
    PV(tk-2), hiding the ~900ns exp latency; one "filler" matmul (next
    chunk's projections / previous chunk's out-projection) is pumped into
    the PE queue per attention tile so the PE never idles and the HAM
    clock gate stays at 8/8 (2.4 GHz).
  - Warmup dummy matmuls during the initial input-DMA wait warm the HAM
    before real work; chunk-0 x is DMA'd in 4 bands so the first
    projection chain starts ~1.5us in.
  - Softmax denominators: bf16 accumulation on DVE, ones-matmul
    broadcast on PE (gpsimd is unusable: Q7 lib reload per op switch,
    4.4us per custom op), reciprocal_approx_fast + tensor_mul divides.
  - RoPE: pre-rotation values kept in f16 (reference casts to bf16 only
    AFTER rotation - quantizing before it was the dominant error source);
    half-swap via a f16 permutation matmul on PE; bias-add+PSUM-read on
    DVE tensor_scalar_add (keeps ACT exp-only); cos/sin tables in f16.
  - Causal diagonal at 128-column granularity (tri mask multiply on DVE).
  - V projection computes V^T (wv stationary), XBAR transpose-DMA into
    [s,d] tiles for the PV matmul.
  - Output stored f16; host accumulates partials in f64.
"""

import numpy as np

for _p in ("/opt/trn_rl_repo", "/root/.axon_site/_ro/trn_rl_repo"):
    import sys

    if _p not in sys.path:
        sys.path.insert(0, _p)

import ml_dtypes
from collections import deque
from contextlib import ExitStack

import concourse.bass as bass
import concourse.bass_isa as bass_isa
import concourse.mybir as mybir
import concourse.tile as tile
from concourse import bacc
from concourse.bass_utils import run_bass_kernel_spmd

F32 = mybir.dt.float32
BF16 = mybir.dt.bfloat16
F16 = mybir.dt.float16
T = 2048
E = 2048
HD = 128
NQH = 4          # q heads per core
KT = E // 128    # 16 k-tiles over embed
NC = T // 512    # 4 512-chunks over time
SCALE = float(E) ** -0.5

_program = None
LAST_EXEC_NS = None
LAST_TRACE = None
LAST_PROFILE_JSON = None


def _build_program():
    nc = bacc.Bacc("TRN2", target_bir_lowering=False, debug=False, num_devices=8)
    # Big inputs stay row-major: the resulting many-small-descriptor gather
    # DMAs scatter across all 16 DMA engines (a single contiguous transfer
    # serializes onto one engine at ~60GB/s - measured).
    xT_d = nc.declare_dram_parameter("xT", [E, T], F16, isOutput=False)
    wq_d = nc.declare_dram_parameter("wq", [E, NQH * HD], F16, isOutput=False)
    wk_d = nc.declare_dram_parameter("wk", [E, HD], F16, isOutput=False)
    wv_d = nc.declare_dram_parameter("wv", [E, HD], F16, isOutput=False)
    wo_d = nc.declare_dram_parameter("wo", [NQH * HD, E], BF16, isOutput=False)
    ct_d = nc.declare_dram_parameter("ct", [HD, T], F16, isOutput=False)
    st_d = nc.declare_dram_parameter("st", [HD, T], F16, isOutput=False)
    tri_d = nc.declare_dram_parameter("tri", [HD, HD], BF16, isOutput=False)
    psw_d = nc.declare_dram_parameter("psw", [HD, HD], F16, isOutput=False)
    bq_d = nc.declare_dram_parameter("bq", [HD, NQH], F32, isOutput=False)
    bk_d = nc.declare_dram_parameter("bk", [HD, 1], F32, isOutput=False)
    out_d = nc.declare_dram_parameter("out", [T, E], F16, isOutput=True)

    with tile.TileContext(nc) as tc, ExitStack() as ctx:
        consts = ctx.enter_context(tc.tile_pool(name="consts", bufs=1))
        rope = ctx.enter_context(tc.tile_pool(name="rope", bufs=2))
        vsp = ctx.enter_context(tc.tile_pool(name="vsp", bufs=2))
        ptp = ctx.enter_context(tc.tile_pool(name="ptp", bufs=4))
        accp = ctx.enter_context(tc.tile_pool(name="accp", bufs=2))
        rinvp = ctx.enter_context(tc.tile_pool(name="rinvp", bufs=2))
        otp = ctx.enter_context(tc.tile_pool(name="otp", bufs=12))
        outp = ctx.enter_context(tc.tile_pool(name="outp", bufs=2))
        # PSUM: pss 3 + scr(qsw/psb) 1 + psot 2 + psP 2 = 8 banks exactly
        psS = ctx.enter_context(tc.tile_pool(name="psS", bufs=3, space=bass.MemorySpace.PSUM))
        psOT = ctx.enter_context(tc.tile_pool(name="psOT", bufs=2, space=bass.MemorySpace.PSUM))
        psP = ctx.enter_context(tc.tile_pool(name="psP", bufs=2, space=bass.MemorySpace.PSUM))

        # ---- persistent tiles ---------------------------------------------
        wk = consts.tile([128, KT * HD], F16, tag="wk", name="wk")
        wv = consts.tile([128, KT * HD], F16, tag="wv", name="wv")
        wq = consts.tile([128, KT * NQH * HD], F16, tag="wq", name="wq")
        wo = consts.tile([128, NQH * E], BF16, tag="wo", name="wo")
        xb0 = [consts.tile([128, 4 * 512], F16, tag=f"xb0_{b}", name=f"xb0_{b}")
               for b in range(4)]
        xtc = [None] + [consts.tile([128, KT * 512], F16, tag=f"xtc{c}", name=f"xtc{c}")
                        for c in range(1, NC)]
        ctc = [consts.tile([128, 512], F16, tag=f"ctc{c}", name=f"ctc{c}")
               for c in range(NC)]
        stc = [consts.tile([128, 512], F16, tag=f"stc{c}", name=f"stc{c}")
               for c in range(NC)]
        tri = consts.tile([128, 128], BF16, tag="tri", name="tri")
        psw = consts.tile([128, 128], F16, tag="psw", name="psw")
        bq_t = consts.tile([HD, NQH], F32, tag="bq", name="bq_t")
        bk_t = consts.tile([HD, 1], F32, tag="bk", name="bk_t")
        zeros = consts.tile([128, 512], BF16, tag="zeros", name="zeros")
        ones128 = consts.tile([128, 128], BF16, tag="ones", name="ones128")
        nc.vector.memset(ones128[:], 1.0)

        qTc = [[consts.tile([128, 512], BF16, tag=f"qT{h}_{c}", name=f"qT{h}_{c}")
                for c in range(NC)] for h in range(NQH)]
        kTc = [consts.tile([128, 512], BF16, tag=f"kT{c}", name=f"kT{c}")
               for c in range(NC)]
        vA = [consts.tile([128, 128], BF16, tag=f"vA{t}", name=f"vA{t}")
              for t in range(4 * NC)]

        def xsl(c, k):
            # k-tile [128, 512] of chunk c's transposed x
            if c == 0:
                return xb0[k // 4][:, (k % 4) * 512:(k % 4 + 1) * 512]
            return xtc[c][:, k * 512:(k + 1) * 512]

        def split_rows(src_ap, p=128):
            # [(k p), f] -> [p, k, f]: one DMA that deposits each 128-row
            # band k into its own column block of the destination tile.
            return src_ap.rearrange("(k p) f -> p k f", p=p)

        # ---- PE warmup (HAM un-throttle) before input DMAs land -----------
        nc.vector.memset(zeros[:], 0.0)

        def warm_mm():
            warm = psP.tile([128, 512], F32, tag="psP", name="warm")
            nc.tensor.matmul(warm[:], zeros[:, 0:128], zeros[:], start=True, stop=True)

        for i in range(10):
            warm_mm()

        # ---- input DMA issue order (first-needed first) -------------------
        # Per-queue DMAs serialize (queue occupancy ~ transfer time), so:
        #   sync   = chunk-0 x bands + runtime vA transposes (latency-critical)
        #   gpsimd = small tables, then xtc1-3, wo, then runtime out DMAs
        #   scalar = wq alone (q proj needs it ~15us in)
        nc.scalar.dma_start(wq[:], split_rows(wq_d[:, :]))
        nc.gpsimd.dma_start(wk[:], split_rows(wk_d[:, :]))
        nc.gpsimd.dma_start(wv[:], split_rows(wv_d[:, :]))
        nc.gpsimd.dma_start(ctc[0][:], ct_d[:, 0:512])
        nc.gpsimd.dma_start(stc[0][:], st_d[:, 0:512])
        nc.gpsimd.dma_start(psw[:], psw_d[:, :])
        nc.gpsimd.dma_start(bk_t[:], bk_d[:, :])
        nc.gpsimd.dma_start(bq_t[:], bq_d[:, :])
        nc.gpsimd.dma_start(tri[:], tri_d[:, :])
        for b in range(4):
            nc.sync.dma_start(xb0[b][:], split_rows(xT_d[b * 512:(b + 1) * 512, 0:512]))
        for c in range(1, NC):
            nc.gpsimd.dma_start(ctc[c][:], ct_d[:, c * 512:(c + 1) * 512])
            nc.gpsimd.dma_start(stc[c][:], st_d[:, c * 512:(c + 1) * 512])
        for c in range(1, NC):
            nc.gpsimd.dma_start(xtc[c][:], split_rows(xT_d[:, c * 512:(c + 1) * 512]))
        nc.gpsimd.dma_start(wo[:], split_rows(wo_d[:, :]))

        # ---- filler scheduler ---------------------------------------------
        prereq = deque()    # proj units for the NEXT chunk: must drain before it
        optional = deque()  # outproj units: drain opportunistically

        def pump(n):
            k = 0
            while k < n:
                q = prereq if prereq else optional
                if not q:
                    return k
                try:
                    next(q[0])
                    k += 1
                except StopIteration:
                    q.popleft()
            return k

        def drain(q):
            while q:
                try:
                    next(q[0])
                except StopIteration:
                    q.popleft()

        # ---- projection units (generators yielding after each PE instr) ---
        def gen_proj(c, w_col, dst, bias_ap, kind, warm_pad=0):
            ps = psP.tile([128, 512], F32, tag="psP", name=f"ps_{kind}{c}")
            for k in range(KT):
                nc.tensor.matmul(ps[:], w_col(k), xsl(c, k),
                                 start=(k == 0), stop=(k == KT - 1))
                if warm_pad and k < KT - 1:
                    # keep HAM warm while DMA-paced: dep-free dummy matmuls
                    # into the scratch bank (free during chunk-0 V proj)
                    for _ in range(warm_pad):
                        wt = psS.tile([128, 512], F32, tag="scr", bufs=1,
                                      name="warmp")
                        nc.tensor.matmul(wt[:], zeros[:, 0:128], zeros[:],
                                         start=True, stop=True)
                yield
            if kind == "v":
                vsb = vsp.tile([128, 512], BF16, tag="vsb", name="vsb")
                nc.scalar.copy(vsb[:], ps[:])
                for tt in range(4):
                    nc.sync.dma_start(
                        vA[4 * c + tt][:], vsb[:, tt * 128:(tt + 1) * 128],
                        transpose=True)
            else:
                # rope: qsb = ps + bias (f16, on DVE); half-swap on PE;
                # dst = qsb*cos (pool) + qsw*sin (dve), summed on pool.
                qsb = rope.tile([128, 512], F16, tag="qsb", name="qsb")
                nc.vector.tensor_scalar_add(qsb[:], ps[:], bias_ap)
                qsw = psS.tile([128, 512], F32, tag="scr", bufs=1, name="qsw")
                nc.tensor.matmul(qsw[:], psw[:], qsb[:], start=True, stop=True)
                yield
                t1 = rope.tile([128, 512], F32, tag="t1", name="t1")
                nc.vector.tensor_mul(t1[:], qsb[:], ctc[c][:])
                t2 = rope.tile([128, 512], F32, tag="t2", name="t2")
                nc.vector.tensor_mul(t2[:], qsw[:], stc[c][:])
                nc.vector.tensor_add(dst[:], t1[:], t2[:])

        def enqueue_proj(c):
            # V first: its transpose-DMA chain has the longest latency tail
            prereq.append(gen_proj(
                c, lambda k: wv[:, k * HD:(k + 1) * HD], None, None, "v",
                warm_pad=2 if c == 0 else 0))
            prereq.append(gen_proj(
                c, lambda k: wk[:, k * HD:(k + 1) * HD], kTc[c], bk_t[:, 0:1], "k"))
            for h in range(NQH):
                prereq.append(gen_proj(
                    c,
                    lambda k, h=h: wq[:, k * 512 + h * HD:k * 512 + (h + 1) * HD],
                    qTc[h][c], bq_t[:, h:h + 1], "q"))

        # ---- out-projection units -----------------------------------------
        def gen_outproj(qc, ots):
            for i in range(4):
                osb = outp.tile([128, E], F16, tag="osb", name="osb")
                rows = slice((qc * 4 + i) * 128, (qc * 4 + i + 1) * 128)
                for e in range(4):
                    psf = psP.tile([128, 512], F32, tag="psP", name="psf")
                    for h in range(NQH):
                        nc.tensor.matmul(
                            psf[:], ots[h][:, i * 128:(i + 1) * 128],
                            wo[:, h * E + e * 512:h * E + (e + 1) * 512],
                            start=(h == 0), stop=(h == NQH - 1))
                        yield
                    esl = slice(e * 512, (e + 1) * 512)
                    if e % 2 == 0:
                        nc.vector.tensor_copy(osb[:, esl], psf[:])
                    else:
                        nc.scalar.copy(osb[:, esl], psf[:])
                    nc.gpsimd.dma_start(out_d[rows, esl], osb[:, esl])

        # ---- attention for one 512-query chunk (per head, 3-deep pipe) ----
        def attention(qc):
            Tt = 4 * qc + 4
            ots = []
            for h in range(NQH):
                psot = psOT.tile([128, 512], F32, tag="psot", name="psot")
                acc = accp.tile([128, 512], BF16, tag="acc", name="acc")
                pts = [None] * Tt

                def emit_qk(tk, h=h, acc=acc, pts=pts):
                    pss = psS.tile([128, 512], F32, tag="pss", name="pss")
                    pt = ptp.tile([128, 512], BF16, tag="pt", name="pt")
                    if tk < 4 * qc:
                        nc.tensor.matmul(
                            pss[:], kTc[tk // 4][:, (tk % 4) * 128:(tk % 4 + 1) * 128],
                            qTc[h][qc][:], start=True, stop=True)
                        nc.scalar.activation(
                            pt[:], pss[:], mybir.ActivationFunctionType.Exp)
                        if tk == 0:
                            nc.vector.tensor_copy(acc[:], pt[:])
                        else:
                            nc.vector.tensor_add(acc[:], acc[:], pt[:])
                    else:
                        j = tk - 4 * qc
                        sl = slice(j * 128, 512)
                        dsl = slice(j * 128, (j + 1) * 128)
                        nc.tensor.matmul(
                            pss[:, sl], kTc[qc][:, j * 128:(j + 1) * 128],
                            qTc[h][qc][:, sl], start=True, stop=True)
                        nc.scalar.activation(
                            pt[:, sl], pss[:, sl], mybir.ActivationFunctionType.Exp)
                        nc.vector.tensor_mul(pt[:, dsl], pt[:, dsl], tri[:])
                        if tk == 0:
                            nc.vector.tensor_copy(acc[:], pt[:])
                        else:
                            nc.vector.tensor_add(acc[:, sl], acc[:, sl], pt[:, sl])
                    pts[tk] = pt

                def emit_pv(tk, psot=psot, pts=pts):
                    pt = pts[tk]
                    start = (tk == 0)
                    stop = (tk == Tt - 1)
                    if tk < 4 * qc:
                        nc.tensor.matmul(psot[:], vA[tk][:], pt[:],
                                         start=start, stop=stop)
                    else:
                        j = tk - 4 * qc
                        sl = slice(j * 128, 512)
                        nc.tensor.matmul(psot[:, sl], vA[tk][:], pt[:, sl],
                                         start=start, stop=stop)
                    pts[tk] = None

                emit_qk(0)
                if Tt > 1:
                    emit_qk(1)
                for tk in range(2, Tt):
                    emit_qk(tk)
                    pump(1)
                    emit_pv(tk - 2)
                pump(1)
                if Tt > 1:
                    emit_pv(Tt - 2)
                pump(1)
                emit_pv(Tt - 1)

                # finalize head: denominator (PE ones-matmul broadcast) + divide
                psb = psS.tile([128, 512], F32, tag="scr", bufs=1, name="psb")
                nc.tensor.matmul(psb[:], ones128[:], acc[:], start=True, stop=True)
                rinv = rinvp.tile([128, 512], F32, tag="rinv", name="rinv")
                nc.vector.reciprocal_approx_fast(out=rinv[:], in_=psb[:])
                ot = otp.tile([128, 512], BF16, tag="ot", name="ot")
                nc.vector.tensor_mul(ot[:], psot[:], rinv[:])
                ots.append(ot)
            return ots

        # ---- schedule ------------------------------------------------------
        # chunk-0 projections run in the open (nothing to hide them under)
        enqueue_proj(0)
        drain(prereq)
        for qc in range(NC):
            if qc + 1 < NC:
                enqueue_proj(qc + 1)
            ots = attention(qc)
            optional.append(gen_outproj(qc, ots))
            drain(prereq)
        drain(optional)
    nc.compile()
    return nc


def _rope_tables():
    # quirk: freq exponent uses full n_embed then slices to head_dim//2
    freqs = 10000.0 ** (-(np.arange(0, E, 2, dtype=np.float64) / E))[:HD // 2]
    t = np.arange(T, dtype=np.float64)
    ang = np.outer(freqs, t)                      # [64, T]
    ct = np.empty((HD, T), np.float32)
    st = np.empty((HD, T), np.float32)
    ct[:64] = np.cos(ang)
    ct[64:] = np.cos(ang)
    st[:64] = -np.sin(ang)
    st[64:] = np.sin(ang)
    return ct, st


def kernel(x, Wq, bq, Wk, bk, Wv, bv, Wo, bo):
    global _program, LAST_EXEC_NS, LAST_TRACE, LAST_PROFILE_JSON
    x = np.asarray(x, np.float32)
    Wq, bq = np.asarray(Wq, np.float32), np.asarray(bq, np.float32)
    Wk, bk = np.asarray(Wk, np.float32), np.asarray(bk, np.float32)
    Wv, bv = np.asarray(Wv, np.float32), np.asarray(bv, np.float32)
    Wo, bo = np.asarray(Wo, np.float32), np.asarray(bo, np.float32)
    bf = ml_dtypes.bfloat16

    if _program is None:
        _program = _build_program()

    perm = np.concatenate([np.arange(0, HD, 2), np.arange(1, HD, 2)])
    ct, st = _rope_tables()
    tri = (np.arange(128)[None, :] >= np.arange(128)[:, None]).astype(np.float32)
    psw = np.zeros((128, 128), np.float32)
    psw[(np.arange(128) + 64) % 128, np.arange(128)] = 1.0

    xT = [np.ascontiguousarray(x[b].T).astype(np.float16) for b in range(2)]
    in_maps = []
    for c in range(8):
        b, g = divmod(c, 4)
        qcols = np.concatenate([(4 * g + h) * HD + perm for h in range(NQH)])
        kcols = g * HD + perm
        vcols = np.arange(g * HD, (g + 1) * HD)
        in_maps.append({
            "xT": xT[b],
            "wq": Wq[:, qcols].astype(np.float16),
            "wk": Wk[:, kcols].astype(np.float16),
            "wv": Wv[:, vcols].astype(np.float16),
            "wo": (Wo[g * 512:(g + 1) * 512, :] * SCALE).astype(bf),
            "ct": ct.astype(np.float16),
            "st": st.astype(np.float16),
            "tri": tri.astype(bf),
            "psw": psw.astype(np.float16),
            "bq": np.ascontiguousarray(
                bq[np.concatenate([(4 * g + h) * HD + perm for h in range(NQH)])]
                .reshape(NQH, HD).T).astype(np.float32),
            "bk": bk[kcols].reshape(HD, 1).astype(np.float32),
        })

    import time
    t0 = time.time()
    res = run_bass_kernel_spmd(_program, in_maps, list(range(8)))
    t1 = time.time()
    LAST_EXEC_NS = res.exec_time_ns
    if res.instructions_and_trace is not None:
        LAST_TRACE = res.instructions_and_trace[1]
    LAST_PROFILE_JSON = res.profile_json
    if LAST_EXEC_NS is None:
        LAST_EXEC_NS = int((t1 - t0) * 1e9)  # wall time incl. H2D (upper bound)

    out = np.zeros((2, T, E), np.float64)
    for c in range(8):
        out[c // 4] += np.asarray(res.results[c]["out"], np.float64)
    # bv folded: after softmax each row sums to 1, scaled by SCALE inside Wo
    obias = np.repeat(bv.astype(np.float64).reshape(4, HD), 4, axis=0).reshape(-1)
    bo_eff = bo.astype(np.float64) + SCALE * (obias @ Wo.astype(np.float64))
    out += bo_eff[None, None, :]
    return out.astype(np.float32)


# revision 36
# speedup vs baseline: 1.2274x; 1.1922x over previous
"""GQA attention block (B=2,T=2048,E=2048,H=16,KV=4) on 8 trn2 NeuronCores.

Sharding: core c -> batch b=c//4, kv-group g=c%4 (q-heads 4g..4g+3, kv head g).
Each core computes its 4 heads end-to-end plus the partial output projection
(Wo rows for its heads); host sums the 4 partials per batch and adds bias.

v4 schedule/layout (single continuous PE stream):
  - 3-deep software-pipelined attention: QK(tk) runs two tiles ahead of
    PV(tk-2), hiding the ~900ns exp latency; one "filler" matmul (next
    chunk's projections / previous chunk's out-projection) is pumped into
    the PE queue per attention tile so the PE never idles and the HAM
    clock gate stays at 8/8 (2.4 GHz).
  - Warmup dummy matmuls during the initial input-DMA wait warm the HAM
    before real work; chunk-0 x is DMA'd in 4 bands so the first
    projection chain starts ~1.5us in.
  - Softmax denominators: bf16 accumulation on DVE, ones-matmul
    broadcast on PE (gpsimd is unusable: Q7 lib reload per op switch,
    4.4us per custom op), reciprocal_approx_fast + tensor_mul divides.
  - RoPE: pre-rotation values kept in f16 (reference casts to bf16 only
    AFTER rotation - quantizing before it was the dominant error source);
    half-swap via a f16 permutation matmul on PE; bias-add+PSUM-read on
    DVE tensor_scalar_add (keeps ACT exp-only); cos/sin tables in f16.
  - Causal diagonal at 128-column granularity (tri mask multiply on DVE).
  - V projection computes V^T (wv stationary), XBAR transpose-DMA into
    [s,d] tiles for the PV matmul.
  - Output stored f16; host accumulates partials in f64.
"""

import numpy as np

for _p in ("/opt/trn_rl_repo", "/root/.axon_site/_ro/trn_rl_repo"):
    import sys

    if _p not in sys.path:
        sys.path.insert(0, _p)

import ml_dtypes
from collections import deque
from contextlib import ExitStack

import concourse.bass as bass
import concourse.bass_isa as bass_isa
import concourse.mybir as mybir
import concourse.tile as tile
from concourse import bacc
from concourse.bass_utils import run_bass_kernel_spmd

F32 = mybir.dt.float32
BF16 = mybir.dt.bfloat16
F16 = mybir.dt.float16
T = 2048
E = 2048
HD = 128
NQH = 4          # q heads per core
KT = E // 128    # 16 k-tiles over embed
NC = T // 512    # 4 512-chunks over time
SCALE = float(E) ** -0.5

_program = None
LAST_EXEC_NS = None
LAST_TRACE = None
LAST_PROFILE_JSON = None


def _build_program():
    nc = bacc.Bacc("TRN2", target_bir_lowering=False, debug=False, num_devices=8)
    # Big inputs stay row-major: the resulting many-small-descriptor gather
    # DMAs scatter across all 16 DMA engines (a single contiguous transfer
    # serializes onto one engine at ~60GB/s - measured).
    xT_d = nc.declare_dram_parameter("xT", [E, T], F16, isOutput=False)
    wq_d = nc.declare_dram_parameter("wq", [E, NQH * HD], F16, isOutput=False)
    wk_d = nc.declare_dram_parameter("wk", [E, HD], F16, isOutput=False)
    wv_d = nc.declare_dram_parameter("wv", [E, HD], F16, isOutput=False)
    wo_d = nc.declare_dram_parameter("wo", [NQH * HD, E], BF16, isOutput=False)
    ct_d = nc.declare_dram_parameter("ct", [HD, T], F16, isOutput=False)
    st_d = nc.declare_dram_parameter("st", [HD, T], F16, isOutput=False)
    tri_d = nc.declare_dram_parameter("tri", [HD, HD], BF16, isOutput=False)
    psw_d = nc.declare_dram_parameter("psw", [HD, HD], F16, isOutput=False)
    bq_d = nc.declare_dram_parameter("bq", [HD, NQH], F32, isOutput=False)
    bk_d = nc.declare_dram_parameter("bk", [HD, 1], F32, isOutput=False)
    out_d = nc.declare_dram_parameter("out", [T, E], F16, isOutput=True)

    with tile.TileContext(nc) as tc, ExitStack() as ctx:
        consts = ctx.enter_context(tc.tile_pool(name="consts", bufs=1))
        rope = ctx.enter_context(tc.tile_pool(name="rope", bufs=2))
        vsp = ctx.enter_context(tc.tile_pool(name="vsp", bufs=2))
        ptp = ctx.enter_context(tc.tile_pool(name="ptp", bufs=4))
        accp = ctx.enter_context(tc.tile_pool(name="accp", bufs=2))
        rinvp = ctx.enter_context(tc.tile_pool(name="rinvp", bufs=2))
        otp = ctx.enter_context(tc.tile_pool(name="otp", bufs=12))
        outp = ctx.enter_context(tc.tile_pool(name="outp", bufs=2))
        # PSUM: pss 3 + scr(qsw/psb) 1 + psot 2 + psP 2 = 8 banks exactly
        psS = ctx.enter_context(tc.tile_pool(name="psS", bufs=3, space=bass.MemorySpace.PSUM))
        psOT = ctx.enter_context(tc.tile_pool(name="psOT", bufs=2, space=bass.MemorySpace.PSUM))
        psP = ctx.enter_context(tc.tile_pool(name="psP", bufs=2, space=bass.MemorySpace.PSUM))

        # ---- persistent tiles ---------------------------------------------
        wk = consts.tile([128, KT * HD], F16, tag="wk", name="wk")
        wv = consts.tile([128, KT * HD], F16, tag="wv", name="wv")
        wq = consts.tile([128, KT * NQH * HD], F16, tag="wq", name="wq")
        wo = consts.tile([128, NQH * E], BF16, tag="wo", name="wo")
        xb0 = [consts.tile([128, 4 * 512], F16, tag=f"xb0_{b}", name=f"xb0_{b}")
               for b in range(4)]
        xtc = [None] + [consts.tile([128, KT * 512], F16, tag=f"xtc{c}", name=f"xtc{c}")
                        for c in range(1, NC)]
        ctc = [consts.tile([128, 512], F16, tag=f"ctc{c}", name=f"ctc{c}")
               for c in range(NC)]
        stc = [consts.tile([128, 512], F16, tag=f"stc{c}", name=f"stc{c}")
               for c in range(NC)]
        tri = consts.tile([128, 128], BF16, tag="tri", name="tri")
        psw = consts.tile([128, 128], F16, tag="psw", name="psw")
        bq_t = consts.tile([HD, NQH], F32, tag="bq", name="bq_t")
        bk_t = consts.tile([HD, 1], F32, tag="bk", name="bk_t")
        zeros = consts.tile([128, 512], BF16, tag="zeros", name="zeros")
        ones128 = consts.tile([128, 128], BF16, tag="ones", name="ones128")
        nc.vector.memset(ones128[:], 1.0)

        qTc = [[consts.tile([128, 512], BF16, tag=f"qT{h}_{c}", name=f"qT{h}_{c}")
                for c in range(NC)] for h in range(NQH)]
        kTc = [consts.tile([128, 512], BF16, tag=f"kT{c}", name=f"kT{c}")
               for c in range(NC)]
        vA = [consts.tile([128, 128], BF16, tag=f"vA{t}", name=f"vA{t}")
              for t in range(4 * NC)]

        def xsl(c, k):
            # k-tile [128, 512] of chunk c's transposed x
            if c == 0:
                return xb0[k // 4][:, (k % 4) * 512:(k % 4 + 1) * 512]
            return xtc[c][:, k * 512:(k + 1) * 512]

        def split_rows(src_ap, p=128):
            # [(k p), f] -> [p, k, f]: one DMA that deposits each 128-row
            # band k into its own column block of the destination tile.
            return src_ap.rearrange("(k p) f -> p k f", p=p)

        # ---- PE warmup (HAM un-throttle) before input DMAs land -----------
        nc.vector.memset(zeros[:], 0.0)

        def warm_mm():
            warm = psP.tile([128, 512], F32, tag="psP", name="warm")
            nc.tensor.matmul(warm[:], zeros[:, 0:128], zeros[:], start=True, stop=True)

        for i in range(10):
            warm_mm()

        # ---- input DMA issue order (first-needed first) -------------------
        # Per-queue DMAs serialize (queue occupancy ~ transfer time), so:
        #   sync   = chunk-0 x bands + runtime vA transposes (latency-critical)
        #   gpsimd = small tables, then xtc1-3, wo, then runtime out DMAs
        #   scalar = wq alone (q proj needs it ~15us in)
        nc.scalar.dma_start(wq[:], split_rows(wq_d[:, :]))
        nc.gpsimd.dma_start(wk[:], split_rows(wk_d[:, :]))
        nc.gpsimd.dma_start(wv[:], split_rows(wv_d[:, :]))
        nc.gpsimd.dma_start(ctc[0][:], ct_d[:, 0:512])
        nc.gpsimd.dma_start(stc[0][:], st_d[:, 0:512])
        nc.gpsimd.dma_start(psw[:], psw_d[:, :])
        nc.gpsimd.dma_start(bk_t[:], bk_d[:, :])
        nc.gpsimd.dma_start(bq_t[:], bq_d[:, :])
        nc.gpsimd.dma_start(tri[:], tri_d[:, :])
        for b in range(4):
            nc.sync.dma_start(xb0[b][:], split_rows(xT_d[b * 512:(b + 1) * 512, 0:512]))
        for c in range(1, NC):
            nc.gpsimd.dma_start(ctc[c][:], ct_d[:, c * 512:(c + 1) * 512])
            nc.gpsimd.dma_start(stc[c][:], st_d[:, c * 512:(c + 1) * 512])
        for c in range(1, NC):
            nc.gpsimd.dma_start(xtc[c][:], split_rows(xT_d[:, c * 512:(c + 1) * 512]))
        nc.gpsimd.dma_start(wo[:], split_rows(wo_d[:, :]))

        # ---- filler scheduler ---------------------------------------------
        prereq = deque()    # proj units for the NEXT chunk: must drain before it
        optional = deque()  # outproj units: drain opportunistically

        def pump(n):
            k = 0
            while k < n:
                q = prereq if prereq else optional
                if not q:
                    return k
                try:
                    next(q[0])
                    k += 1
                except StopIteration:
                    q.popleft()
            return k

        def drain(q):
            while q:
                try:
                    next(q[0])
                except StopIteration:
                    q.popleft()

        # ---- projection units (generators yielding after each PE instr) ---
        def gen_proj(c, w_col, dst, bias_ap, kind, warm_pad=0):
            ps = psP.tile([128, 512], F32, tag="psP", name=f"ps_{kind}{c}")
            for k in range(KT):
                nc.tensor.matmul(ps[:], w_col(k), xsl(c, k),
                                 start=(k == 0), stop=(k == KT - 1))
                if warm_pad and k < KT - 1:
                    # keep HAM warm while DMA-paced: dep-free dummy matmuls
                    # into the scratch bank (free during chunk-0 V proj)
                    for _ in range(warm_pad):
                        wt = psS.tile([128, 512], F32, tag="scr", bufs=1,
                                      name="warmp")
                        nc.tensor.matmul(wt[:], zeros[:, 0:128], zeros[:],
                                         start=True, stop=True)
                yield
            if kind == "v":
                vsb = vsp.tile([128, 512], BF16, tag="vsb", name="vsb")
                nc.scalar.copy(vsb[:], ps[:])
                for tt in range(4):
                    nc.sync.dma_start(
                        vA[4 * c + tt][:], vsb[:, tt * 128:(tt + 1) * 128],
                        transpose=True)
            else:
                # rope: qsb = ps + bias (f16, on DVE); half-swap on PE;
                # dst = qsb*cos (pool) + qsw*sin (dve), summed on pool.
                qsb = rope.tile([128, 512], F16, tag="qsb", name="qsb")
                nc.vector.tensor_scalar_add(qsb[:], ps[:], bias_ap)
                qsw = psS.tile([128, 512], F32, tag="scr", bufs=1, name="qsw")
                nc.tensor.matmul(qsw[:], psw[:], qsb[:], start=True, stop=True)
                yield
                t1 = rope.tile([128, 512], F32, tag="t1", name="t1")
                nc.vector.tensor_mul(t1[:], qsb[:], ctc[c][:])
                t2 = rope.tile([128, 512], F32, tag="t2", name="t2")
                nc.vector.tensor_mul(t2[:], qsw[:], stc[c][:])
                nc.vector.tensor_add(dst[:], t1[:], t2[:])

        def enqueue_proj(c):
            # V first: its transpose-DMA chain has the longest latency tail
            prereq.append(gen_proj(
                c, lambda k: wv[:, k * HD:(k + 1) * HD], None, None, "v",
                warm_pad=2 if c == 0 else 0))
            prereq.append(gen_proj(
                c, lambda k: wk[:, k * HD:(k + 1) * HD], kTc[c], bk_t[:, 0:1], "k"))
            for h in range(NQH):
                prereq.append(gen_proj(
                    c,
                    lambda k, h=h: wq[:, k * 512 + h * HD:k * 512 + (h + 1) * HD],
                    qTc[h][c], bq_t[:, h:h + 1], "q"))

        # ---- out-projection units -----------------------------------------
        def gen_outproj(qc, ots):
            for i in range(4):
                osb = outp.tile([128, E], F16, tag="osb", name="osb")
                rows = slice((qc * 4 + i) * 128, (qc * 4 + i + 1) * 128)
                for e in range(4):
                    psf = psP.tile([128, 512], F32, tag="psP", name="psf")
                    for h in range(NQH):
                        nc.tensor.matmul(
                            psf[:], ots[h][:, i * 128:(i + 1) * 128],
                            wo[:, h * E + e * 512:h * E + (e + 1) * 512],
                            start=(h == 0), stop=(h == NQH - 1))
                        yield
                    esl = slice(e * 512, (e + 1) * 512)
                    if e % 2 == 0:
                        nc.vector.tensor_copy(osb[:, esl], psf[:])
                    else:
                        nc.scalar.copy(osb[:, esl], psf[:])
                    eng = nc.gpsimd if i % 2 == 0 else nc.sync
                    eng.dma_start(out_d[rows, esl], osb[:, esl])

        # ---- attention for one 512-query chunk (per head, 3-deep pipe) ----
        def attention(qc):
            Tt = 4 * qc + 4
            ots = []
            for h in range(NQH):
                psot = psOT.tile([128, 512], F32, tag="psot", name="psot")
                acc = accp.tile([128, 512], BF16, tag="acc", name="acc")
                pts = [None] * Tt

                def emit_qk(tk, h=h, acc=acc, pts=pts):
                    pss = psS.tile([128, 512], F32, tag="pss", name="pss")
                    pt = ptp.tile([128, 512], BF16, tag="pt", name="pt")
                    if tk < 4 * qc:
                        nc.tensor.matmul(
                            pss[:], kTc[tk // 4][:, (tk % 4) * 128:(tk % 4 + 1) * 128],
                            qTc[h][qc][:], start=True, stop=True)
                        nc.scalar.activation(
                            pt[:], pss[:], mybir.ActivationFunctionType.Exp)
                        if tk == 0:
                            nc.vector.tensor_copy(acc[:], pt[:])
                        else:
                            nc.vector.tensor_add(acc[:], acc[:], pt[:])
                    else:
                        j = tk - 4 * qc
                        sl = slice(j * 128, 512)
                        dsl = slice(j * 128, (j + 1) * 128)
                        nc.tensor.matmul(
                            pss[:, sl], kTc[qc][:, j * 128:(j + 1) * 128],
                            qTc[h][qc][:, sl], start=True, stop=True)
                        nc.scalar.activation(
                            pt[:, sl], pss[:, sl], mybir.ActivationFunctionType.Exp)
                        nc.vector.tensor_mul(pt[:, dsl], pt[:, dsl], tri[:])
                        if tk == 0:
                            nc.vector.tensor_copy(acc[:], pt[:])
                        else:
                            nc.vector.tensor_add(acc[:, sl], acc[:, sl], pt[:, sl])
                    pts[tk] = pt

                def emit_pv(tk, psot=psot, pts=pts):
                    pt = pts[tk]
                    start = (tk == 0)
                    stop = (tk == Tt - 1)
                    if tk < 4 * qc:
                        nc.tensor.matmul(psot[:], vA[tk][:], pt[:],
                                         start=start, stop=stop)
                    else:
                        j = tk - 4 * qc
                        sl = slice(j * 128, 512)
                        nc.tensor.matmul(psot[:, sl], vA[tk][:], pt[:, sl],
                                         start=start, stop=stop)
                    pts[tk] = None

                emit_qk(0)
                if Tt > 1:
                    emit_qk(1)
                for tk in range(2, Tt):
                    emit_qk(tk)
                    pump(1)
                    emit_pv(tk - 2)
                pump(1)
                if Tt > 1:
                    emit_pv(Tt - 2)
                pump(1)
                emit_pv(Tt - 1)

                # finalize head: denominator (PE ones-matmul broadcast) + divide
                psb = psS.tile([128, 512], F32, tag="scr", bufs=1, name="psb")
                nc.tensor.matmul(psb[:], ones128[:], acc[:], start=True, stop=True)
                rinv = rinvp.tile([128, 512], F32, tag="rinv", name="rinv")
                nc.vector.reciprocal_approx_fast(out=rinv[:], in_=psb[:])
                ot = otp.tile([128, 512], BF16, tag="ot", name="ot")
                nc.vector.tensor_mul(ot[:], psot[:], rinv[:])
                ots.append(ot)
            return ots

        # ---- schedule ------------------------------------------------------
        # chunk-0 projections run in the open (nothing to hide them under)
        enqueue_proj(0)
        drain(prereq)
        for qc in range(NC):
            if qc + 1 < NC:
                enqueue_proj(qc + 1)
            ots = attention(qc)
            optional.append(gen_outproj(qc, ots))
            drain(prereq)
        drain(optional)
    nc.compile()
    return nc


def _rope_tables():
    # quirk: freq exponent uses full n_embed then slices to head_dim//2
    freqs = 10000.0 ** (-(np.arange(0, E, 2, dtype=np.float64) / E))[:HD // 2]
    t = np.arange(T, dtype=np.float64)
    ang = np.outer(freqs, t)                      # [64, T]
    ct = np.empty((HD, T), np.float32)
    st = np.empty((HD, T), np.float32)
    ct[:64] = np.cos(ang)
    ct[64:] = np.cos(ang)
    st[:64] = -np.sin(ang)
    st[64:] = np.sin(ang)
    return ct, st


def kernel(x, Wq, bq, Wk, bk, Wv, bv, Wo, bo):
    global _program, LAST_EXEC_NS, LAST_TRACE, LAST_PROFILE_JSON
    x = np.asarray(x, np.float32)
    Wq, bq = np.asarray(Wq, np.float32), np.asarray(bq, np.float32)
    Wk, bk = np.asarray(Wk, np.float32), np.asarray(bk, np.float32)
    Wv, bv = np.asarray(Wv, np.float32), np.asarray(bv, np.float32)
    Wo, bo = np.asarray(Wo, np.float32), np.asarray(bo, np.float32)
    bf = ml_dtypes.bfloat16

    if _program is None:
        _program = _build_program()

    perm = np.concatenate([np.arange(0, HD, 2), np.arange(1, HD, 2)])
    ct, st = _rope_tables()
    tri = (np.arange(128)[None, :] >= np.arange(128)[:, None]).astype(np.float32)
    psw = np.zeros((128, 128), np.float32)
    psw[(np.arange(128) + 64) % 128, np.arange(128)] = 1.0

    xT = [np.ascontiguousarray(x[b].T).astype(np.float16) for b in range(2)]
    in_maps = []
    for c in range(8):
        b, g = divmod(c, 4)
        qcols = np.concatenate([(4 * g + h) * HD + perm for h in range(NQH)])
        kcols = g * HD + perm
        vcols = np.arange(g * HD, (g + 1) * HD)
        in_maps.append({
            "xT": xT[b],
            "wq": Wq[:, qcols].astype(np.float16),
            "wk": Wk[:, kcols].astype(np.float16),
            "wv": Wv[:, vcols].astype(np.float16),
            "wo": (Wo[g * 512:(g + 1) * 512, :] * SCALE).astype(bf),
            "ct": ct.astype(np.float16),
            "st": st.astype(np.float16),
            "tri": tri.astype(bf),
            "psw": psw.astype(np.float16),
            "bq": np.ascontiguousarray(
                bq[np.concatenate([(4 * g + h) * HD + perm for h in range(NQH)])]
                .reshape(NQH, HD).T).astype(np.float32),
            "bk": bk[kcols].reshape(HD, 1).astype(np.float32),
        })

    import time
    t0 = time.time()
    res = run_bass_kernel_spmd(_program, in_maps, list(range(8)))
    t1 = time.time()
    LAST_EXEC_NS = res.exec_time_ns
    if res.instructions_and_trace is not None:
        LAST_TRACE = res.instructions_and_trace[1]
    LAST_PROFILE_JSON = res.profile_json
    if LAST_EXEC_NS is None:
        LAST_EXEC_NS = int((t1 - t0) * 1e9)  # wall time incl. H2D (upper bound)

    out = np.zeros((2, T, E), np.float64)
    for c in range(8):
        out[c // 4] += np.asarray(res.results[c]["out"], np.float64)
    # bv folded: after softmax each row sums to 1, scaled by SCALE inside Wo
    obias = np.repeat(bv.astype(np.float64).reshape(4, HD), 4, axis=0).reshape(-1)
    bo_eff = bo.astype(np.float64) + SCALE * (obias @ Wo.astype(np.float64))
    out += bo_eff[None, None, :]
    return out.astype(np.float32)
